# revision 3
# baseline (speedup 1.0000x reference)
"""Causal self-attention on 8 TRN2 NeuronCores.

Sharding: batch (2) x head-groups (4 heads each) -> 8 cores. Each core
computes qkv projection for its 4 heads, causal attention (lower-triangular
128-blocks only), and a partial o-projection. Host concatenates attention
weights, sums o partials, and adds the output bias.

Self-contained: hardcodes B=2, T=2048, C=1024, H=16, D=64.
"""

import numpy as np
from contextlib import ExitStack

import concourse.bass as bass
import concourse.tile as tile
import concourse.mybir as mybir
from concourse import bacc
import concourse.bass_utils as bass_utils

F32 = mybir.dt.float32
F32R = mybir.dt.float32r
AF = mybir.ActivationFunctionType
ALU = mybir.AluOpType

P = 128
T = 2048
C = 1024
D = 64
HL = 4          # local heads per core
NQKV = 3 * HL * D   # 768
TB = T // P     # 16 row blocks
CB = C // P     # 8 contraction chunks for qkv
MB = NQKV // P  # 6 qkv output row-blocks
TCH = T // 512  # 4 column chunks of 512


def build_nc():
    nc = bacc.Bacc("TRN2", target_bir_lowering=False, debug=False)

    x = nc.dram_tensor("x", [T, C], F32, kind="ExternalInput").ap()
    w3 = nc.dram_tensor("w3", [C, NQKV], F32, kind="ExternalInput").ap()
    b3 = nc.dram_tensor("b3", [NQKV], F32, kind="ExternalInput").ap()
    wo = nc.dram_tensor("wo", [HL * D, C], F32, kind="ExternalInput").ap()
    ident_in = nc.dram_tensor("ident", [P, P], F32, kind="ExternalInput").ap()
    id2_in = nc.dram_tensor("id2", [P, D], F32, kind="ExternalInput").ap()
    mask_in = nc.dram_tensor("mask", [P, P], F32, kind="ExternalInput").ap()

    attn4 = nc.dram_tensor("attn4", [HL, T, T], F32, kind="ExternalOutput").ap()
    o_part = nc.dram_tensor("o_part", [T, C], F32, kind="ExternalOutput").ap()

    with ExitStack() as ctx:
        tc = ctx.enter_context(tile.TileContext(nc))
        const = ctx.enter_context(tc.tile_pool(name="const", bufs=1))
        persist = ctx.enter_context(tc.tile_pool(name="persist", bufs=1))
        small = ctx.enter_context(tc.tile_pool(name="small", bufs=8))
        pool_mm = ctx.enter_context(tc.tile_pool(name="pmm", bufs=2, space="PSUM"))
        pool_tr = ctx.enter_context(tc.tile_pool(name="ptr", bufs=2, space="PSUM"))
        pool_av = ctx.enter_context(tc.tile_pool(name="pav", bufs=2, space="PSUM"))

        # ---- constants ----
        ident = const.tile([P, P], F32)
        nc.sync.dma_start(ident[:], ident_in)
        id2 = const.tile([P, D], F32)
        nc.sync.dma_start(id2[:], id2_in)
        id2_r = const.tile([P, D], F32R)
        nc.vector.tensor_copy(id2_r[:], id2[:])
        mask = const.tile([P, P], F32)
        nc.sync.dma_start(mask[:], mask_in)

        b_sb = const.tile([P, MB], F32)
        nc.sync.dma_start(b_sb[:], b3.rearrange("(o p) -> p o", p=P))

        # rounded weights (fp32r matmul operands must be compute-produced)
        w_r = persist.tile([P, CB, NQKV], F32R)
        wo_r = persist.tile([P, 2, C], F32R)
        with tc.tile_pool(name="wtmp", bufs=2) as wtmp:
            for cb in range(CB):
                t_ = wtmp.tile([P, NQKV], F32, tag="wld")
                nc.sync.dma_start(t_[:], w3[cb * P:(cb + 1) * P, :])
                nc.any.tensor_copy(w_r[:, cb, :], t_[:])
            for g in range(2):
                t_ = wtmp.tile([P, C], F32, tag="wold")
                nc.sync.dma_start(t_[:], wo[g * P:(g + 1) * P, :])
                nc.any.tensor_copy(wo_r[:, g, :], t_[:])

        qkvT = persist.tile([P, MB, T], F32R)   # [q0q1 | k0k1 | v0v1] row-blocks
        v_sb = persist.tile([P, TB, HL * D], F32R)
        avT = persist.tile([P, 2, T], F32R)

        # ---- phase AB: transpose x and project to qkvT ----
        with (
            tc.tile_pool(name="xld", bufs=3) as pool_x,
            tc.tile_pool(name="xtc", bufs=2) as pool_xt,
        ):
            for tch in range(TCH):
                xT_chunk = pool_xt.tile([P, CB, 512], F32R)
                for tbl in range(4):
                    tb = tch * 4 + tbl
                    x_tb = pool_x.tile([P, C], F32)
                    nc.sync.dma_start(x_tb[:], x[tb * P:(tb + 1) * P, :])
                    for cbp in range(2):   # pack 4 transposes per psum bank
                        pst = pool_tr.tile([P, 512], F32, tag="tr")
                        for k in range(4):
                            cb = cbp * 4 + k
                            nc.tensor.transpose(
                                pst[:, k * P:(k + 1) * P],
                                x_tb[:, cb * P:(cb + 1) * P],
                                ident[:],
                            )
                        for k in range(4):
                            cb = cbp * 4 + k
                            nc.any.tensor_copy(
                                xT_chunk[:, cb, tbl * P:(tbl + 1) * P],
                                pst[:, k * P:(k + 1) * P],
                            )
                for mb in range(MB):
                    psq = pool_mm.tile([P, 512], F32, tag="mm")
                    for cb in range(CB):
                        nc.tensor.matmul(
                            psq[:],
                            w_r[:, cb, mb * P:(mb + 1) * P],
                            xT_chunk[:, cb, :],
                            start=(cb == 0),
                            stop=(cb == CB - 1),
                        )
                    nc.scalar.activation(
                        qkvT[:, mb, tch * 512:(tch + 1) * 512], psq[:],
                        AF.Identity, bias=b_sb[:, mb:mb + 1], scale=1.0,
                    )

        # ---- phase V: v rows of qkvT -> natural layout ----
        for h in range(HL):
            base = D * (h % 2)
            vT_h = qkvT[base:base + D, 4 + h // 2, :]
            for jbp in range(2):    # pack 8 transposes per psum bank
                psv = pool_tr.tile([P, 512], F32R, tag="tr")
                for k in range(8):
                    jb = jbp * 8 + k
                    nc.tensor.transpose(
                        psv[:, k * D:(k + 1) * D],
                        vT_h[:, jb * P:(jb + 1) * P],
                        id2_r[base:base + D, :],
                        tile_position=(base, 0),
                    )
                for k in range(8):
                    jb = jbp * 8 + k
                    nc.any.tensor_copy(
                        v_sb[:, jb, h * D:(h + 1) * D],
                        psv[:, k * D:(k + 1) * D],
                    )

        # ---- phase C: attention per head ----
        with tc.tile_pool(name="e", bufs=6) as pool_e, \
             tc.tile_pool(name="eT", bufs=3) as pool_eT:
            for h in range(HL):
                base = D * (h % 2)
                qT_h = qkvT[base:base + D, h // 2, :]
                kT_h = qkvT[base:base + D, 2 + h // 2, :]
                for c in range(4):
                    e_tiles = []
                    for i0 in range(4 * c, 4 * c + 4):
                        j_end = (i0 + 1) * P
                        nch = (j_end + 511) // 512
                        e = pool_e.tile([P, T], F32, tag="e")
                        accs = small.tile([P, 4], F32, tag="accs")
                        for jc in range(nch):
                            n = min(512, j_end - jc * 512)
                            pss = pool_mm.tile([P, 512], F32, tag="mm")
                            nc.tensor.matmul(
                                pss[:, :n],
                                qT_h[:, i0 * P:(i0 + 1) * P],
                                kT_h[:, jc * 512:jc * 512 + n],
                                start=True, stop=True,
                            )
                            if jc == nch - 1:
                                nc.vector.tensor_tensor(
                                    pss[:, n - P:n], pss[:, n - P:n], mask[:],
                                    ALU.add,
                                )
                            nc.scalar.activation(
                                e[:, jc * 512:jc * 512 + n], pss[:, :n],
                                AF.Exp, scale=0.125,
                                accum_out=accs[:, jc:jc + 1],
                            )
                        recip = small.tile([P, 1], F32, tag="recip")
                        if nch == 1:
                            nc.vector.reciprocal(recip[:], accs[:, 0:1])
                        else:
                            ssum = small.tile([P, 1], F32, tag="ssum")
                            nc.vector.tensor_reduce(
                                ssum[:], accs[:, :nch],
                                mybir.AxisListType.X, ALU.add,
                            )
                            nc.vector.reciprocal(recip[:], ssum[:])
                        nc.vector.tensor_scalar_mul(
                            e[:, :j_end], e[:, :j_end], recip[:]
                        )
                        nc.sync.dma_start(
                            attn4[h, i0 * P:(i0 + 1) * P, 0:j_end],
                            e[:, :j_end],
                        )
                        e_tiles.append(e)
                    # attn_v for this 512-wide i-chunk
                    psav = pool_av.tile([D, 512], F32, tag="av")
                    for jb in range(4 * c + 4):
                        eT = pool_eT.tile([P, 512], F32R, tag="eT")
                        i0lo = max(4 * c, jb)
                        off = (i0lo - 4 * c) * P
                        pstr_t = pool_tr.tile([P, 512], F32, tag="tr")
                        if off > 0:
                            nc.vector.memset(pstr_t[:, :off], 0.0)
                        for i0 in range(i0lo, 4 * c + 4):
                            oo = (i0 - 4 * c) * P
                            nc.tensor.transpose(
                                pstr_t[:, oo:oo + P],
                                e_tiles[i0 - 4 * c][:, jb * P:(jb + 1) * P],
                                ident[:],
                            )
                        nc.scalar.copy(eT[:], pstr_t[:])
                        nc.tensor.matmul(
                            psav[:],
                            v_sb[:, jb, h * D:(h + 1) * D],
                            eT[:],
                            start=(jb == 0), stop=(jb == 4 * c + 3),
                        )
                    nc.any.tensor_copy(
                        avT[base:base + D, h // 2, c * 512:(c + 1) * 512],
                        psav[:],
                    )

        # ---- phase D: o projection (partial; host adds bias + reduces) ----
        with tc.tile_pool(name="osb", bufs=3) as pool_o:
            for tb in range(TB):
                for ncol in range(2):
                    pso = pool_mm.tile([P, 512], F32, tag="mm")
                    for g in range(2):
                        nc.tensor.matmul(
                            pso[:],
                            avT[:, g, tb * P:(tb + 1) * P],
                            wo_r[:, g, ncol * 512:(ncol + 1) * 512],
                            start=(g == 0), stop=(g == 1),
                        )
                    o_sb = pool_o.tile([P, 512], F32, tag="osb")
                    nc.any.tensor_copy(o_sb[:], pso[:])
                    nc.sync.dma_start(
                        o_part[tb * P:(tb + 1) * P, ncol * 512:(ncol + 1) * 512],
                        o_sb[:],
                    )

    nc.compile()
    return nc


_NC_CACHE = []


def _get_nc():
    if not _NC_CACHE:
        _NC_CACHE.append(build_nc())
    return _NC_CACHE[0]


def _host_consts():
    ident = np.eye(P, dtype=np.float32)
    id2 = np.concatenate([np.eye(D, dtype=np.float32)] * 2, axis=0)
    mask = np.triu(np.full((P, P), -1e30, dtype=np.float32), 1)
    return ident, id2, mask


def kernel(x, w_qkv, b_qkv, w_o, b_o, _trace=False, _trace_kwargs=None):
    x = np.ascontiguousarray(np.asarray(x, dtype=np.float32))
    w_qkv = np.asarray(w_qkv, dtype=np.float32)
    b_qkv = np.asarray(b_qkv, dtype=np.float32)
    w_o = np.asarray(w_o, dtype=np.float32)
    b_o = np.asarray(b_o, dtype=np.float32)

    H = 16
    ident, id2, mask = _host_consts()
    in_maps = []
    for core in range(8):
        b = core // 4
        hg = (core % 4) * HL
        cols = np.r_[hg * D:(hg + HL) * D]
        w3 = np.concatenate(
            [w_qkv[:, cols], w_qkv[:, C + cols], w_qkv[:, 2 * C + cols]], axis=1
        )
        b3 = np.concatenate(
            [b_qkv[cols], b_qkv[C + cols], b_qkv[2 * C + cols]]
        )
        in_maps.append({
            "x": np.ascontiguousarray(x[b]),
            "w3": np.ascontiguousarray(w3),
            "b3": np.ascontiguousarray(b3),
            "wo": np.ascontiguousarray(w_o[hg * D:(hg + HL) * D, :]),
            "ident": ident,
            "id2": id2,
            "mask": mask,
        })

    nc = _get_nc()
    kw = {}
    if _trace:
        kw = dict(trace=True, **(_trace_kwargs or {}))
    res = bass_utils.run_bass_kernel_spmd(
        nc, in_maps, core_ids=list(range(8)), **kw
    )

    attn_w = np.empty((2, H, T, T), dtype=np.float32)
    o = np.zeros((2, T, C), dtype=np.float32)
    for core in range(8):
        b = core // 4
        hg = (core % 4) * HL
        r = res.results[core]
        attn_w[b, hg:hg + HL] = r["attn4"]
        o[b] += r["o_part"]
    o += b_o

    if _trace:
        return (o, attn_w), res
    return o, attn_w
